# revision 32
# baseline (speedup 1.0000x reference)
"""Distributed causal multi-head attention block for 8 TRN2 NeuronCores.

Problem: y = proj(softmax_causal((x Wq)(x Wk)^T / 8) (x Wv)) with
B=1, S=4096, D=1024, H=16 heads, Dh=64, all float32.

Sharding (head-parallel attention + query-row-split projection):
- Each core c owns heads {2c, 2c+1}: it projects the FULL sequence through
  its 128 columns of Wq/Wk/Wv (x is replicated, transposed on host), runs
  causal attention for its two heads over all 4096 queries, and normalizes
  by the softmax denominator on the producer side.
- The attention output is re-sharded head-major -> query-major with FOUR
  chunk-pair AllToAlls (one per pair of 512-query chunks), issued as soon
  as each pair's attention completes so the collectives overlap compute.
  Slot k of each A2A carries the 64-query row block destined for core k,
  so core c ends up owning rows [64c, 64c+64) of every 512-query chunk.
- Each core projects its 64-row blocks through the full Wproj; the host
  interleaves the 8 per-core outputs back into [4096, 1024].

Compute dtypes: everything in bf16 with fp32 PSUM accumulation. The scalar
engine is the serial bottleneck (H*S^2/2 exp elements/core at 1
elem/cycle/lane), so the schedule keeps it saturated: scores for key-tile
kt+2 are matmul'd while kt is exp'd, and all other PE work (QKV projection
of the next chunk, output projection of completed A2A groups) is woven into
the PE slack of the attention loop. Softmax uses no max-subtraction (scores
are O(4) for this operator's weight scale) and folds the denominator via an
appended ones-column in V; the reciprocal of the denominator row is
computed via a 32x32 stream-transpose so the DVE iterative-divide runs over
32 free elements instead of 512.
"""

import sys

sys.path.insert(0, "/opt/trn_rl_repo")

import numpy as np
import ml_dtypes

from concourse import bacc, tile, mybir
from concourse import bass_utils
from concourse.bass_utils import run_bass_kernel_spmd

bass_utils.upload_artifacts = lambda tmpdir: tmpdir  # no S3 in this container

dt = mybir.dt
AF = mybir.ActivationFunctionType

N_CORES = 8
S = 4096
D = 1024
P = 128
CH = 512            # seq chunk (query block per iteration)
NCHUNK = S // CH    # 8
KT_PER_CH = CH // P  # 4
# A2A groups: chunk pairs early (fewer collectives), singles at the end so
# the last collective + projection tail is as short as possible.
GROUPS = [(0, 1), (2, 3), (4, 5), (6,), (7,)]
CHUNK_GROUP = {c: (g, gi.index(c)) for g, gi in enumerate(GROUPS) for c in gi}

_BUILD_CACHE = {}
DEBUG_TAPS = False


def _build(has_bq: bool, has_bp: bool):
    key = (has_bq, has_bp)
    if key in _BUILD_CACHE:
        return _BUILD_CACHE[key]

    nc = bacc.Bacc("TRN2", target_bir_lowering=False, debug=False,
                   num_devices=N_CORES)

    f32, bf16 = dt.float32, dt.bfloat16

    # ---- external I/O (per-core values supplied via in_maps) ----
    xT_ext = nc.dram_tensor("xT", [NCHUNK, NCHUNK, P, CH], bf16, kind="ExternalInput")
    wq_ext = nc.dram_tensor("wq", [NCHUNK, P, P], bf16, kind="ExternalInput")
    wk_ext = nc.dram_tensor("wk", [NCHUNK, P, P], bf16, kind="ExternalInput")
    wv_ext = nc.dram_tensor("wv", [NCHUNK, P, P], bf16, kind="ExternalInput")
    wp_ext = nc.dram_tensor("wp", [NCHUNK, P, D], bf16, kind="ExternalInput")
    bq_ext = nc.dram_tensor("bq", [P, 3], f32, kind="ExternalInput")
    bp_ext = nc.dram_tensor("bp", [1, D], f32, kind="ExternalInput")
    bv_ext = nc.dram_tensor("bv", [1, P], bf16, kind="ExternalInput")
    out_ext = nc.dram_tensor("out", [NCHUNK, 64, D], f32, kind="ExternalOutput")
    dbg_ext = (nc.dram_tensor("dbg", [6, P, 2 * CH], bf16, kind="ExternalOutput")
               if DEBUG_TAPS else None)

    # ---- inline constants ----
    # one shared 128x128 diagonal triangle mask: mask[k, q] = 1 if k <= q.
    # With the diagonal key-tile's queries restricted to [128j, 512), the
    # same triangle applies to the first 128 queries of every diagonal tile.
    kk = np.arange(P)[:, None]
    qq = np.arange(P)[None, :]
    masks_np = (kk <= qq).astype(ml_dtypes.bfloat16)   # [128, 128]
    masks_dram = nc.inline_tensor(masks_np, name="masks_const")
    ones_np = np.ones((P, P), dtype=np.float32)
    ones_dram = nc.inline_tensor(ones_np, name="ones_const")

    with tile.TileContext(nc) as tc:
        with tc.tile_pool(name="const", bufs=1) as const, \
             tc.tile_pool(name="wpool", bufs=1) as wpool, \
             tc.tile_pool(name="resid", bufs=1) as resid, \
             tc.tile_pool(name="xp", bufs=2) as xp, \
             tc.tile_pool(name="probs", bufs=4) as probsp, \
             tc.tile_pool(name="small", bufs=2) as smallp, \
             tc.tile_pool(name="attnp", bufs=4) as attnp, \
             tc.tile_pool(name="atp", bufs=2) as atp, \
             tc.tile_pool(name="outp", bufs=4) as outpool, \
             tc.tile_pool(name="psA", bufs=2, space="PSUM") as psA, \
             tc.tile_pool(name="psS", bufs=2, space="PSUM") as psS, \
             tc.tile_pool(name="psV", bufs=2, space="PSUM") as psV, \
             tc.tile_pool(name="dram", bufs=1, space="DRAM") as dram:

            # ---- constants ----
            if has_bp:
                ones_r_sb = const.tile([1, P], f32)
                nc.sync.dma_start(ones_r_sb[:], ones_dram.ap()[0:1, :])
                bp_sb = const.tile([1, D], f32)
                nc.sync.dma_start(bp_sb[:], bp_ext.ap())
            if has_bq:
                bq_sb = const.tile([P, 3], f32)
                nc.sync.dma_start(bq_sb[:], bq_ext.ap())
                ones_bf_sb = const.tile([1, P], bf16)
                nc.vector.memset(ones_bf_sb[:], 1.0)
                bv_sb = const.tile([1, P], bf16)
                nc.sync.dma_start(bv_sb[:], bv_ext.ap())

            # ---- resident weights ----
            wq_sb = wpool.tile([P, NCHUNK, P], bf16)
            wk_sb = wpool.tile([P, NCHUNK, P], bf16)
            wv_sb = wpool.tile([P, NCHUNK, P], bf16)
            nc.sync.dma_start(wq_sb[:], wq_ext.ap().rearrange("t p c -> p t c"))
            nc.sync.dma_start(wk_sb[:], wk_ext.ap().rearrange("t p c -> p t c"))
            masks_sb = const.tile([P, P], bf16)

            # ---- resident per-chunk [Q^T | K^T] (bf16) and V (natural) ----
            qkt_tiles = []  # [128, 1024]: cols 0:512 Q^T, 512:1024 K^T
            v_tiles = []    # per chunk: [128, 4, 130]: per ktile cols 0:64 head-a V,
                            # 64 ones, 65:129 head-b V, 129 ones
            for c in range(NCHUNK):
                qkt_tiles.append(resid.tile([P, 2 * CH], bf16, name=f"qkt{c}"))
                v_tiles.append(resid.tile([P, KT_PER_CH, 130], bf16,
                                          name=f"v{c}"))

            # A2A bounce buffers, one per group (64 query cols per chunk)
            a2a_in = [dram.tile([N_CORES, P, 64 * len(gi)], bf16,
                                name=f"a2a_in{g}")
                      for g, gi in enumerate(GROUPS)]
            a2a_out = [dram.tile([N_CORES, P, 64 * len(gi)], bf16,
                                 name=f"a2a_out{g}")
                       for g, gi in enumerate(GROUPS)]

            x_tiles_all = {}

            # ================= phase A (QKV projection) as filler items ====
            def emit_x_load(c):
                xt = xp.tile([P, NCHUNK, CH], bf16, tag="x", name=f"x{c}")
                nc.sync.dma_start(xt[:], xT_ext.ap()[c].rearrange(
                    "t p q -> p t q"))
                x_tiles_all[c] = xt

            def aq_items(c):
                """Q^T projection for chunk c -> list of closures."""
                state = {}

                def mk_mm(t0, t1):
                    def run():
                        if t0 == 0:
                            state["ps"] = psA.tile([P, CH], f32, tag="qkv",
                                                   name=f"psq{c}")
                        ps = state["ps"]
                        xt = x_tiles_all[c]
                        for t in (t0, t1):
                            nc.tensor.matmul(ps[:], wq_sb[:, t, :],
                                             xt[:, t, :],
                                             start=(t == 0),
                                             stop=(t == NCHUNK - 1))
                    return run

                def evict():
                    ps = state.pop("ps")
                    if has_bq:
                        nc.scalar.activation(qkt_tiles[c][:, 0:CH], ps[:],
                                             AF.Copy,
                                             bias=bq_sb[:, 0][:, None])
                    else:
                        nc.vector.tensor_copy(qkt_tiles[c][:, 0:CH], ps[:])
                return [mk_mm(0, 1), mk_mm(2, 3), mk_mm(4, 5), mk_mm(6, 7),
                        evict]

            def kv_items(c):
                """K^T + natural-V projection for chunk c -> closures."""
                state = {}

                def mk_kmm(t0, t1):
                    def run():
                        if t0 == 0:
                            state["ps"] = psA.tile([P, CH], f32, tag="qkv",
                                                   name=f"psk{c}")
                        ps = state["ps"]
                        xt = x_tiles_all[c]
                        for t in (t0, t1):
                            nc.tensor.matmul(ps[:], wk_sb[:, t, :],
                                             xt[:, t, :],
                                             start=(t == 0),
                                             stop=(t == NCHUNK - 1))
                    return run

                def kevict():
                    ps = state.pop("ps")
                    if has_bq:
                        nc.scalar.activation(qkt_tiles[c][:, CH:2 * CH], ps[:],
                                             AF.Copy,
                                             bias=bq_sb[:, 1][:, None])
                    else:
                        nc.vector.tensor_copy(qkt_tiles[c][:, CH:2 * CH],
                                              ps[:])

                def mk_v(b):
                    def run():
                        psv = psA.tile([P, P], f32, tag="qkv",
                                       name=f"psv{c}_{b}")
                        xt = x_tiles_all[c]
                        if has_bq:
                            nc.tensor.matmul(psv[:], ones_bf_sb[0:1, :],
                                             bv_sb[0:1, :], start=True,
                                             stop=False)
                        for t in range(NCHUNK):
                            nc.tensor.matmul(
                                psv[:], xt[:, t, P * b:P * (b + 1)],
                                wv_sb[:, t, :],
                                start=(t == 0 and not has_bq),
                                stop=(t == NCHUNK - 1))
                        nc.vector.tensor_copy(v_tiles[c][:, b, 0:64],
                                              psv[:, 0:64])
                        nc.vector.tensor_copy(v_tiles[c][:, b, 65:129],
                                              psv[:, 64:128])
                    return run

                def vdone():
                    nc.vector.memset(v_tiles[c][:, :, 64:65], 1.0)
                    nc.vector.memset(v_tiles[c][:, :, 129:130], 1.0)
                    x_tiles_all.pop(c)

                return [mk_kmm(0, 1), mk_kmm(2, 3), mk_kmm(4, 5), mk_kmm(6, 7),
                        kevict, mk_v(0), mk_v(1), mk_v(2), mk_v(3), vdone]

            # ================= projection (post-A2A) as filler items =======
            wp_sb = wpool.tile([P, NCHUNK, D], bf16)

            def proj_items(g):
                """Project group g's chunks (my 64 rows each). For pair
                groups both chunks stack into one M=128 matmul."""
                gi = GROUPS[g]
                M = 64 * len(gi)
                state = {}

                def load():
                    at = atp.tile([P, NCHUNK, M], bf16, tag="at",
                                  name=f"at{g}")
                    nc.sync.dma_start(at[:], a2a_out[g][:].rearrange(
                        "k p q -> p k q"))
                    state["at"] = at
                    if DEBUG_TAPS and g == 0:
                        nc.sync.dma_start(dbg_ext.ap()[5, :, 0:NCHUNK * M],
                                          at[:])

                items = [load]

                def mk_mm(dc, t0):
                    def run():
                        if t0 == 0:
                            state[dc] = psA.tile([M, CH], f32, tag="qkv",
                                                 name=f"po{g}_{dc}")
                            if has_bp:
                                nc.tensor.matmul(
                                    state[dc][:], ones_r_sb[0:1, 0:M],
                                    bp_sb[0:1, CH * dc:CH * (dc + 1)],
                                    start=True, stop=False)
                        po = state[dc]
                        at = state["at"]
                        dsl = slice(CH * dc, CH * (dc + 1))
                        for t in range(t0, t0 + 4):
                            nc.tensor.matmul(po[:], at[:, t, :],
                                             wp_sb[:, t, dsl],
                                             start=(t == 0 and not has_bp),
                                             stop=(t == NCHUNK - 1))
                    return run

                def mk_evict(dc):
                    def run():
                        po = state.pop(dc)
                        o_sb = outpool.tile([M, CH], f32, tag="out")
                        nc.vector.tensor_copy(o_sb[:], po[:])
                        for i, c in enumerate(gi):
                            nc.sync.dma_start(
                                out_ext.ap()[c, :, CH * dc:CH * (dc + 1)],
                                o_sb[64 * i:64 * i + 64, :])
                    return run

                for dc in range(2):
                    items.append(mk_mm(dc, 0))
                    items.append(mk_mm(dc, 4))
                    items.append(mk_evict(dc))
                return items

            # ================= attention for one chunk =====================
            def phase_b(c, filler):
                """Causal attention for query chunk c, both heads, with
                `filler` closures woven into the PE slack of the kt loop.

                Diagonal key-tiles (kt = 4c+j) only touch queries
                [128j, 512): sc/exp/av are restricted to that range. Score
                segments per head are packed so each matmul output stays
                within one PSUM bank: seg(h) = 0 / N when 2N <= 512, else
                0 / 512."""
                nkt = KT_PER_CH * (c + 1)
                av = [psV.tile([P, CH], f32, tag="av", name=f"av{c}_{h}")
                      for h in range(2)]

                def geom(kt):
                    j = kt - KT_PER_CH * c
                    qoff = P * j if j >= 0 else 0
                    n = CH - qoff
                    # head 1 always starts at col 512: each matmul output
                    # stays in its own PSUM bank (two start=True groups in
                    # one bank is asking for trouble).
                    return qoff, n, (0, CH)

                def emit_sc(kt):
                    kc, kb = divmod(kt, KT_PER_CH)
                    qoff, n, segs = geom(kt)
                    sc = psS.tile([P, 2 * CH], f32, tag="sc",
                                  name=f"sc{c}_{kt}")
                    for h in range(2):
                        lo, hi = 64 * h, 64 * h + 64
                        nc.tensor.matmul(
                            sc[:, segs[h]:segs[h] + n],
                            qkt_tiles[kc][lo:hi,
                                          CH + P * kb:CH + P * (kb + 1)],
                            qkt_tiles[c][lo:hi, qoff:CH],
                            start=True, stop=True,
                        )
                    return sc

                def emit_exp(kt, sc):
                    qoff, n, segs = geom(kt)
                    width = segs[1] + n
                    pr = probsp.tile([P, 2 * CH], bf16, tag="pr")
                    nc.scalar.activation(pr[:, 0:width], sc[:, 0:width],
                                         AF.Exp, scale=0.125)
                    if kt >= KT_PER_CH * c:
                        for h in range(2):
                            nc.vector.tensor_mul(
                                pr[:, segs[h]:segs[h] + P],
                                pr[:, segs[h]:segs[h] + P],
                                masks_sb[:])
                    return pr

                def emit_av(kt, pr):
                    kc, kb = divmod(kt, KT_PER_CH)
                    qoff, n, segs = geom(kt)
                    for h in range(2):
                        nc.tensor.matmul(
                            av[h][0:65, qoff:CH],
                            v_tiles[kc][:, kb, 65 * h:65 * h + 65],
                            pr[:, segs[h]:segs[h] + n],
                            start=(kt == 0), stop=(kt == nkt - 1),
                        )

                scs = {0: emit_sc(0)}
                if nkt > 1:
                    scs[1] = emit_sc(1)
                nfill = len(filler)
                for kt in range(nkt):
                    pr = emit_exp(kt, scs.pop(kt))
                    if kt + 2 < nkt:
                        scs[kt + 2] = emit_sc(kt + 2)
                    emit_av(kt, pr)
                    # weave filler into the PE slack left by the exp latency
                    want = (nfill * (kt + 1)) // nkt
                    while nfill - len(filler) < want:
                        filler.pop(0)()
                while filler:
                    filler.pop(0)()
                return av

            # ================= normalize + ship one chunk ==================
            def normalize_and_ship(c, av):
                g, half = CHUNK_GROUP[c]
                avs = []
                for h in range(2):
                    a = attnp.tile([65, CH], f32, tag="avs")
                    nc.vector.tensor_copy(a[:], av[h][0:65, :])
                    avs.append(a)
                # denominator rows land on partitions 0 and 32 (32-aligned
                # engine access), so one strided reciprocal covers both heads
                # with free size 16 instead of 512 (DVE divide is iterative).
                dn = smallp.tile([64, CH], f32, tag="dn")
                for h in range(2):
                    nc.vector.tensor_copy(dn[32 * h:32 * h + 1, :],
                                          avs[h][64:65, :])
                tr = smallp.tile([64, CH], f32, tag="tr")
                nc.vector.transpose(tr[:], dn[:])
                rt = smallp.tile([64, CH], f32, tag="rt")
                nc.vector.reciprocal(
                    rt[:].rearrange("p (k j) -> p k j", j=32)[:, :, 0:1],
                    tr[:].rearrange("p (k j) -> p k j", j=32)[:, :, 0:1])
                rb2 = smallp.tile([64, CH], f32, tag="rb2")
                nc.vector.transpose(rb2[:], rt[:])
                # partition_broadcast reads partition 0 only: stage head-b's
                # recip row (at partition 32 after the transpose) to row 0.
                rfix = smallp.tile([1, CH], f32, tag="rfix")
                nc.vector.tensor_copy(rfix[:], rb2[32:33, :])
                for h in range(2):
                    rb = smallp.tile([64, CH], f32, tag="rb")
                    nc.gpsimd.partition_broadcast(
                        rb[:], rfix[:] if h else rb2[0:1, :])
                    attn = attnp.tile([64, CH], bf16, tag="attn")
                    nc.vector.tensor_mul(attn[:], avs[h][0:64, :], rb[:])
                    nc.sync.dma_start(
                        a2a_in[g][:, 64 * h:64 * h + 64,
                                  64 * half:64 * half + 64]
                        .rearrange("k p q -> p k q"),
                        attn[:].rearrange("p (k q) -> p k q", k=N_CORES))
                    if DEBUG_TAPS and c == 0:
                        nc.sync.dma_start(
                            dbg_ext.ap()[3, 64 * h:64 * h + 64, 0:CH],
                            attn[:])
                if DEBUG_TAPS and c == 0:
                    rbb = attnp.tile([64, CH], bf16, tag="attn")
                    nc.vector.tensor_copy(rbb[:], rb2[:])
                    nc.sync.dma_start(dbg_ext.ap()[4, 0:64, 0:CH], rbb[:])

            # ================= main schedule ===============================
            emit_x_load(0)
            nc.sync.dma_start(masks_sb[:], masks_dram.ap())
            nc.sync.dma_start(wv_sb[:], wv_ext.ap().rearrange("t p c -> p t c"))
            for it in aq_items(0):
                it()
            nc.sync.dma_start(wp_sb[:], wp_ext.ap().rearrange("t p c -> p t c"))
            if DEBUG_TAPS:
                nc.sync.dma_start(dbg_ext.ap()[2],
                                  x_tiles_all[0][:, 0:2, :])
            for it in kv_items(0):
                it()
            if DEBUG_TAPS:
                nc.sync.dma_start(dbg_ext.ap()[0], qkt_tiles[0][:])
                nc.sync.dma_start(dbg_ext.ap()[1, :, 0:4 * 130],
                                  v_tiles[0][:])

            # projection of group g is woven into chunk PROJ_AT[g]
            PROJ_AT = {0: 3, 1: 5, 2: 6, 3: 7}
            for c in range(NCHUNK):
                filler = []
                if c + 1 < NCHUNK:
                    emit_x_load(c + 1)
                    filler += aq_items(c + 1)
                    filler += kv_items(c + 1)
                for g, pc in PROJ_AT.items():
                    if pc == c:
                        if not filler:
                            # delay the A2A-gated load past the chunk start
                            filler += [lambda: None] * 10
                        filler += proj_items(g)
                av = phase_b(c, filler)
                normalize_and_ship(c, av)
                g, half = CHUNK_GROUP[c]
                if half == len(GROUPS[g]) - 1:
                    nc.gpsimd.collective_compute(
                        "AllToAll", mybir.AluOpType.bypass,
                        ins=[a2a_in[g][:]], outs=[a2a_out[g][:]],
                        replica_groups=[list(range(N_CORES))],
                    )

            # tail: last group's projection
            for it in proj_items(len(GROUPS) - 1):
                it()

    nc.compile()
    _BUILD_CACHE[key] = nc
    return nc


def _prep_in_maps(x, Wqkv, bqkv, Wproj, bproj):
    x = np.asarray(x, dtype=np.float32)
    Wqkv = np.asarray(Wqkv, dtype=np.float32)
    bqkv = np.asarray(bqkv, dtype=np.float32)
    Wproj = np.asarray(Wproj, dtype=np.float32)
    bproj = np.asarray(bproj, dtype=np.float32)
    xT = np.ascontiguousarray(
        x.reshape(S, D).T.astype(ml_dtypes.bfloat16)
        .reshape(NCHUNK, P, NCHUNK, CH).transpose(2, 0, 1, 3))
    bp = np.ascontiguousarray(bproj.reshape(1, D))
    wp = np.ascontiguousarray(Wproj.astype(ml_dtypes.bfloat16)
                              .reshape(NCHUNK, P, D))
    in_maps = []
    for i in range(N_CORES):
        sl = slice(P * i, P * (i + 1))
        bq = np.stack([bqkv[P * i:P * (i + 1)],
                       bqkv[D + P * i:D + P * (i + 1)],
                       bqkv[2 * D + P * i:2 * D + P * (i + 1)]], axis=1)
        in_maps.append({
            "xT": xT,
            "wq": np.ascontiguousarray(Wqkv[:, sl].astype(ml_dtypes.bfloat16).reshape(NCHUNK, P, P)),
            "wk": np.ascontiguousarray(Wqkv[:, D + P * i:D + P * (i + 1)].astype(ml_dtypes.bfloat16).reshape(NCHUNK, P, P)),
            "wv": np.ascontiguousarray(Wqkv[:, 2 * D + P * i:2 * D + P * (i + 1)].astype(ml_dtypes.bfloat16).reshape(NCHUNK, P, P)),
            "wp": wp,
            "bq": np.ascontiguousarray(bq),
            "bv": bqkv[2 * D + P * i:2 * D + P * (i + 1)].reshape(1, P).astype(ml_dtypes.bfloat16),
            "bp": bp,
        })
    return in_maps


def _run(x, Wqkv, bqkv, Wproj, bproj, trace=False):
    nc = _build(bool(np.any(np.asarray(bqkv))), bool(np.any(np.asarray(bproj))))
    in_maps = _prep_in_maps(x, Wqkv, bqkv, Wproj, bproj)
    res = run_bass_kernel_spmd(nc, in_maps, core_ids=list(range(N_CORES)),
                               trace=trace)
    # core c's out: [NCHUNK, 64, D] = rows [64c, 64c+64) of each chunk
    out = np.empty((NCHUNK, N_CORES, 64, D), dtype=np.float32)
    for c in range(N_CORES):
        out[:, c] = res.results[c]["out"]
    return out.reshape(1, S, D), res


def kernel(x, Wqkv, bqkv, Wproj, bproj):
    out, _ = _run(x, Wqkv, bqkv, Wproj, bproj, trace=False)
    return out
